# revision 3
# baseline (speedup 1.0000x reference)
"""Trainium2 Bass kernel v2: block-diagonal MHA via 32-slot frame layout.

Semantics (fp32 reference):
    q = x@Wq + bq ; k = x@Wk + bk ; v = relu(x@Wv + bv)   (8 heads, d_head=32)
    scores = (q k^T) / sqrt(32) within each 24-token frame, -inf across
    attn = softmax(scores) with +1e-6 in denominator;  out = attn @ v

Mapping: 16 batches data-parallel over 8 cores (2/core).  Within a core the
attention works in a "slot" layout: each 24-token frame occupies a 32-partition
slot (4 frames = 128 partitions = one "quad").  Per (batch, head):
  - scores: 48 tiny matmuls [K=32, M=24, N=24], one per frame, col-tiled so the
    4 frames of a quad run on distinct PE subarrays concurrently.  Output is a
    [128, 12, 24] PSUM tile holding ONLY in-frame scores (75% density) - no
    mask is ever needed.
  - exp: one ScalarE activation over the whole [128, 12, 24] tile.
  - AV: 48 matmuls [K=24, M=24, N=33] against a ones-augmented V in slot
    layout; dead slot rows are never read (K=24).  Column 32 gives the softmax
    denominator per token.
  - normalize: reciprocal + broadcast multiply on DVE/GpSimd.
Projections compute qT/kT (d_model-partitioned, bias via ScalarE) and vT in a
slotted free layout (relu+bias fused in the ScalarE PSUM->SBUF copy), then PE
transposes put V into slot-token partitions.  x is transposed via PE in fp32
and cast to bf16 in the PSUM->SBUF copy, so no standalone cast pass exists.
"""

import math
from contextlib import ExitStack

import numpy as np

import concourse.bass as bass
from concourse import bacc
import concourse.mybir as mybir
import concourse.tile as tile
from concourse.bass_utils import run_bass_kernel_spmd
from concourse.masks import make_identity

F32 = mybir.dt.float32
BF16 = mybir.dt.bfloat16
AF = mybir.ActivationFunctionType
ALU = mybir.AluOpType

BS = 16
SEQ = 48
J = 24            # tokens per frame
N_TOK = SEQ * J   # 1152
D_IN = 256
H = 8
DH = 32
DM = 256
N_CORES = 8
B2 = BS // N_CORES           # batches per core
TOK = B2 * N_TOK             # 2304 tokens per core
QD = 12                      # quads per batch (4 frames each)
NQ = B2 * QD                 # quads per core
FR = 4 * QD                  # frames per batch (48)
CH = 384                     # projection chunk (16 frames)
NCH = N_TOK * B2 // CH       # 6 chunks
SCALE = 1.0 / math.sqrt(DH)
EPS = 1e-6

FP32_TP = True               # fp32 PE transpose for x (cast in copy-out)

_CACHE = {}


def _build(trace_sim=False):
    nc = bacc.Bacc(trn_type="TRN2")

    x_d = nc.dram_tensor("x", [TOK, D_IN], F32, kind="ExternalInput")
    wq_d = nc.dram_tensor("Wq", [D_IN, DM], F32, kind="ExternalInput")
    wk_d = nc.dram_tensor("Wk", [D_IN, DM], F32, kind="ExternalInput")
    wv_d = nc.dram_tensor("Wv", [D_IN, DM], F32, kind="ExternalInput")
    bq_d = nc.dram_tensor("bq", [DM], F32, kind="ExternalInput")
    bk_d = nc.dram_tensor("bk", [DM], F32, kind="ExternalInput")
    bv_d = nc.dram_tensor("bv", [DM], F32, kind="ExternalInput")
    out_d = nc.dram_tensor("out", [TOK, DM], F32, kind="ExternalOutput")

    with tile.TileContext(nc, trace_sim=trace_sim) as tc, ExitStack() as ctx:
        singles = ctx.enter_context(tc.tile_pool(name="singles", bufs=1))
        psproj = ctx.enter_context(tc.tile_pool(name="psproj", bufs=2, space="PSUM"))
        pstp = ctx.enter_context(tc.tile_pool(name="pstp", bufs=2, space="PSUM"))
        pssc = ctx.enter_context(tc.tile_pool(name="pssc", bufs=2, space="PSUM"))
        psav = ctx.enter_context(tc.tile_pool(name="psav", bufs=2, space="PSUM"))
        epool = ctx.enter_context(tc.tile_pool(name="epool", bufs=3))
        rpool = ctx.enter_context(tc.tile_pool(name="rpool", bufs=2))

        # ---- constants ----
        ident = singles.tile([128, 128], BF16)
        make_identity(nc, ident)
        identf = singles.tile([128, 128], F32, tag="identf")
        make_identity(nc, identf)

        w_bf = []
        for wd in (wq_d, wk_d, wv_d):
            wf = singles.tile([128, 2, DM], F32, tag=f"wf_{wd.name}")
            nc.sync.dma_start(wf, wd[:].rearrange("(a p) m -> p a m", p=128))
            wb = singles.tile([128, 2, DM], BF16, tag=f"w_{wd.name}")
            nc.scalar.activation(out=wb, in_=wf, func=AF.Copy)
            w_bf.append(wb)
        wq_bf, wk_bf, wv_bf = w_bf

        # per-partition biases [128, 2 halves]
        b_sb = {}
        for bd in (bq_d, bk_d, bv_d):
            t = singles.tile([128, 2], F32, tag=f"b_{bd.name}")
            nc.sync.dma_start(t, bd[:].rearrange("(a p) -> p a", p=128))
            b_sb[bd.name] = t

        # ---- persistent activations ----
        x_f32 = singles.tile([128, 18, D_IN], F32, tag="x_f32")
        xT = singles.tile([128, 2, TOK], BF16, tag="xT")
        qT = singles.tile([128, 2, TOK], BF16, tag="qT")
        kT = singles.tile([128, 2, TOK], BF16, tag="kT")
        # vT in slotted free layout: [dm-half part, half, frame, 32slot]
        vTs = singles.tile([128, 2, B2 * FR, 32], BF16, tag="vTs")
        # V in slot-token partitions, ones-augmented per head:
        # [128 slot-part, quad, half, h4, 33]
        vsl = singles.tile([128, NQ, 2, 4, DH + 1], BF16, tag="vsl")
        nc.vector.memset(vsl[:, :, :, :, DH:DH + 1], 1.0)
        out_sb = singles.tile([128, B2, QD, DM], F32, tag="out_sb")

        # ---- load x, cast to bf16 (overlapped with DMA), transpose ----
        x_view = x_d[:].rearrange("(t p) d -> p t d", p=128)
        for c in range(9):
            nc.sync.dma_start(
                x_f32[:, 2 * c:2 * (c + 1), :], x_view[:, 2 * c:2 * (c + 1), :]
            )
        x_bf = singles.tile([128, 18, D_IN], BF16, tag="x_bf")
        for t in range(18):
            eng = nc.vector if t % 2 else nc.scalar
            if eng is nc.scalar:
                nc.scalar.activation(
                    out=x_bf[:, t, :], in_=x_f32[:, t, :], func=AF.Copy
                )
            else:
                nc.vector.tensor_copy(x_bf[:, t, :], x_f32[:, t, :])
        # 4 bf16 transposes per PSUM bank, one batched copy per bank
        for t2 in range(9):
            tp = pstp.tile([128, 2, 2, 128], BF16, tag="xtp")
            for dt in range(2):
                for a in range(2):
                    nc.tensor.transpose(
                        tp[:, dt, a],
                        x_bf[:, 2 * t2 + dt, a * 128:(a + 1) * 128],
                        ident,
                    )
            nc.vector.tensor_copy(
                xT[:, :, 256 * t2:256 * (t2 + 1)].rearrange(
                    "p h (dt c) -> p dt h c", dt=2
                ),
                tp.rearrange("p dt h c -> p dt h c"),
            )

        # ---- projections (chunk-outer) with V transposes interleaved ----
        for c6 in range(NCH):
            c0 = c6 * CH
            for half in range(2):
                # q and k -> dense token layout, bias per-partition
                for di, (dst, wb, bn) in enumerate(
                    ((qT, wq_bf, "bq"), (kT, wk_bf, "bk"))
                ):
                    ps = psproj.tile([128, CH], F32, tag="proj")
                    for kk in range(2):
                        nc.tensor.matmul(
                            ps,
                            lhsT=wb[:, kk, half * 128:(half + 1) * 128],
                            rhs=xT[:, kk, c0:c0 + CH],
                            start=(kk == 0),
                            stop=(kk == 1),
                        )
                    if di == 0:
                        nc.scalar.activation(
                            out=dst[:, half, c0:c0 + CH],
                            in_=ps,
                            func=AF.Identity,
                            bias=b_sb[bn][:, half:half + 1],
                            scale=1.0,
                        )
                    else:
                        nc.vector.tensor_scalar_add(
                            dst[:, half, c0:c0 + CH],
                            ps,
                            b_sb[bn][:, half:half + 1],
                        )
                # v -> slotted vT, relu+bias fused
                ps = psproj.tile([128, CH], F32, tag="proj")
                for kk in range(2):
                    nc.tensor.matmul(
                        ps,
                        lhsT=wv_bf[:, kk, half * 128:(half + 1) * 128],
                        rhs=xT[:, kk, c0:c0 + CH],
                        start=(kk == 0),
                        stop=(kk == 1),
                    )
                f0 = c6 * 16
                nc.scalar.activation(
                    out=vTs[:, half, f0:f0 + 16, 0:J],
                    in_=ps.rearrange("p (f j) -> p f j", j=J),
                    func=AF.Relu,
                    bias=b_sb["bv"][:, half:half + 1],
                    scale=1.0,
                )
            # slot transposes for this chunk's 4 quads (2 per psum bank)
            for q2 in range(2):
                bq = 4 * c6 + 2 * q2
                vtp = pstp.tile([128, 2, 2, 128], BF16, tag="xtp")
                for dq in range(2):
                    for half in range(2):
                        nc.tensor.transpose(
                            vtp[:, dq, half],
                            vTs[:, half, 4 * (bq + dq):4 * (bq + dq) + 4, :],
                            ident,
                        )
                nc.vector.tensor_copy(
                    vsl[:, bq:bq + 2, :, :, 0:DH],
                    vtp.rearrange("p dq h (f d) -> p dq h f d", d=DH),
                )

        # ---- attention ----
        out_view = out_d[:].rearrange(
            "(b q s i) d -> b s i q d", b=B2, q=QD, s=4, i=J
        )
        # Software-pipelined: emit scores/exp for step i before AV/normalize
        # for step i-1 so the in-order PE queue never stalls on an exp.
        e_q = []
        for i in range(2 * H + 1):
            if i < 2 * H:
                b, h = divmod(i, H)
                half, h4 = h // 4, h % 4
                hr = 32 * h4
                sc = pssc.tile([128, QD, J], F32, tag="sc")
                if i < 2:
                    nc.vector.memset(sc, 0.0)  # init dead slot rows once/buffer
                for qd in range(QD):
                    for j in range(4):
                        t0 = b * N_TOK + qd * 96 + j * J
                        nc.tensor.matmul(
                            sc[32 * j:32 * j + J, qd, :],
                            lhsT=kT[hr:hr + 32, half, t0:t0 + J],
                            rhs=qT[hr:hr + 32, half, t0:t0 + J],
                            start=True,
                            stop=True,
                            tile_position=(hr, 32 * j),
                        )
                e = epool.tile([128, QD, J], BF16, tag="e")
                nc.scalar.activation(out=e, in_=sc, func=AF.Exp, scale=SCALE)
                e_q.append((b, h, e))
            if i >= 1:
                b, h, e = e_q[i - 1]
                half, h4 = h // 4, h % 4
                av = psav.tile([128, QD, DH + 1], F32, tag="av")
                if i <= 2:
                    nc.vector.memset(av, 0.0)
                for qd in range(QD):
                    for j in range(4):
                        nc.tensor.matmul(
                            av[32 * j:32 * j + J, qd, :],
                            lhsT=e[32 * j:32 * j + J, qd, :],
                            rhs=vsl[32 * j:32 * j + J, b * QD + qd, half, h4, :],
                            start=True,
                            stop=True,
                            tile_position=(32 * j, 32 * j),
                        )
                rt = rpool.tile([128, QD], F32, tag="rt")
                nc.vector.tensor_scalar_add(rt, av[:, :, DH], EPS)
                nc.vector.reciprocal(rt, rt)
                nc.vector.tensor_tensor(
                    out_sb[:, b, :, DH * h:DH * (h + 1)],
                    av[:, :, 0:DH],
                    rt[:, :, None].to_broadcast((128, QD, DH)),
                    ALU.mult,
                )
                if h == H - 1:
                    # fine-grained stores spread work across all DMA queues
                    for s in range(4):
                        for g3 in range(4):
                            nc.sync.dma_start(
                                out_view[b, s, :, 3 * g3:3 * (g3 + 1), :],
                                out_sb[32 * s:32 * s + J, b, 3 * g3:3 * (g3 + 1), :],
                            )

    nc.compile()
    return nc


def _get_nc():
    if "nc" not in _CACHE:
        _CACHE["nc"] = _build()
    return _CACHE["nc"]


def _run(inputs, **kw):
    nc = _get_nc()
    x = np.ascontiguousarray(inputs["x"], dtype=np.float32)
    shared = {
        k: np.ascontiguousarray(inputs[k], dtype=np.float32)
        for k in ("Wq", "Wk", "Wv", "bq", "bk", "bv")
    }
    in_maps = []
    for c in range(N_CORES):
        m = dict(shared)
        m["x"] = np.ascontiguousarray(x[c * B2:(c + 1) * B2].reshape(TOK, D_IN))
        in_maps.append(m)
    res = run_bass_kernel_spmd(nc, in_maps, core_ids=list(range(N_CORES)), **kw)
    out = np.concatenate(
        [r["out"].reshape(B2, N_TOK, DM) for r in res.results], axis=0
    )
    return out, res


def kernel(**inputs) -> np.ndarray:
    out, _ = _run(inputs)
    return out


# revision 4
# speedup vs baseline: 1.0617x; 1.0617x over previous
"""Trainium2 Bass kernel v2: block-diagonal MHA via 32-slot frame layout.

Semantics (fp32 reference):
    q = x@Wq + bq ; k = x@Wk + bk ; v = relu(x@Wv + bv)   (8 heads, d_head=32)
    scores = (q k^T) / sqrt(32) within each 24-token frame, -inf across
    attn = softmax(scores) with +1e-6 in denominator;  out = attn @ v

Mapping: 16 batches data-parallel over 8 cores (2/core).  Within a core the
attention works in a "slot" layout: each 24-token frame occupies a 32-partition
slot (4 frames = 128 partitions = one "quad").  Per (batch, head):
  - scores: 48 tiny matmuls [K=32, M=24, N=24], one per frame, col-tiled so the
    4 frames of a quad run on distinct PE subarrays concurrently.  Output is a
    [128, 12, 24] PSUM tile holding ONLY in-frame scores (75% density) - no
    mask is ever needed.
  - exp: one ScalarE activation over the whole [128, 12, 24] tile.
  - AV: 48 matmuls [K=24, M=24, N=33] against a ones-augmented V in slot
    layout; dead slot rows are never read (K=24).  Column 32 gives the softmax
    denominator per token.
  - normalize: reciprocal + broadcast multiply on DVE/GpSimd.
Projections compute qT/kT (d_model-partitioned, bias via ScalarE) and vT in a
slotted free layout (relu+bias fused in the ScalarE PSUM->SBUF copy), then PE
transposes put V into slot-token partitions.  x is transposed via PE in fp32
and cast to bf16 in the PSUM->SBUF copy, so no standalone cast pass exists.
"""

import math
from contextlib import ExitStack

import numpy as np

import concourse.bass as bass
from concourse import bacc
import concourse.mybir as mybir
import concourse.tile as tile
from concourse.bass_utils import run_bass_kernel_spmd
from concourse.masks import make_identity

F32 = mybir.dt.float32
BF16 = mybir.dt.bfloat16
AF = mybir.ActivationFunctionType
ALU = mybir.AluOpType

BS = 16
SEQ = 48
J = 24            # tokens per frame
N_TOK = SEQ * J   # 1152
D_IN = 256
H = 8
DH = 32
DM = 256
N_CORES = 8
B2 = BS // N_CORES           # batches per core
TOK = B2 * N_TOK             # 2304 tokens per core
QD = 12                      # quads per batch (4 frames each)
NQ = B2 * QD                 # quads per core
FR = 4 * QD                  # frames per batch (48)
CH = 384                     # projection chunk (16 frames)
NCH = N_TOK * B2 // CH       # 6 chunks
SCALE = 1.0 / math.sqrt(DH)
EPS = 1e-6

FP32_TP = True               # fp32 PE transpose for x (cast in copy-out)

_CACHE = {}


def _build(trace_sim=False):
    nc = bacc.Bacc(trn_type="TRN2")

    x_d = nc.dram_tensor("x", [TOK, D_IN], F32, kind="ExternalInput")
    wq_d = nc.dram_tensor("Wq", [D_IN, DM], F32, kind="ExternalInput")
    wk_d = nc.dram_tensor("Wk", [D_IN, DM], F32, kind="ExternalInput")
    wv_d = nc.dram_tensor("Wv", [D_IN, DM], F32, kind="ExternalInput")
    bq_d = nc.dram_tensor("bq", [DM], F32, kind="ExternalInput")
    bk_d = nc.dram_tensor("bk", [DM], F32, kind="ExternalInput")
    bv_d = nc.dram_tensor("bv", [DM], F32, kind="ExternalInput")
    out_d = nc.dram_tensor("out", [TOK, DM], F32, kind="ExternalOutput")

    with tile.TileContext(nc, trace_sim=trace_sim) as tc, ExitStack() as ctx:
        singles = ctx.enter_context(tc.tile_pool(name="singles", bufs=1))
        psproj = ctx.enter_context(tc.tile_pool(name="psproj", bufs=2, space="PSUM"))
        pstp = ctx.enter_context(tc.tile_pool(name="pstp", bufs=2, space="PSUM"))
        pssc = ctx.enter_context(tc.tile_pool(name="pssc", bufs=2, space="PSUM"))
        psav = ctx.enter_context(tc.tile_pool(name="psav", bufs=2, space="PSUM"))
        epool = ctx.enter_context(tc.tile_pool(name="epool", bufs=3))
        rpool = ctx.enter_context(tc.tile_pool(name="rpool", bufs=2))

        # ---- constants ----
        ident = singles.tile([128, 128], BF16)
        make_identity(nc, ident)
        identf = singles.tile([128, 128], F32, tag="identf")
        make_identity(nc, identf)

        w_bf = []
        for wd in (wq_d, wk_d, wv_d):
            wf = singles.tile([128, 2, DM], F32, tag=f"wf_{wd.name}")
            nc.sync.dma_start(wf, wd[:].rearrange("(a p) m -> p a m", p=128))
            wb = singles.tile([128, 2, DM], BF16, tag=f"w_{wd.name}")
            nc.scalar.activation(out=wb, in_=wf, func=AF.Copy)
            w_bf.append(wb)
        wq_bf, wk_bf, wv_bf = w_bf

        # per-partition biases [128, 2 halves]
        b_sb = {}
        for bd in (bq_d, bk_d, bv_d):
            t = singles.tile([128, 2], F32, tag=f"b_{bd.name}")
            nc.sync.dma_start(t, bd[:].rearrange("(a p) -> p a", p=128))
            b_sb[bd.name] = t

        # ---- persistent activations ----
        x_f32 = singles.tile([128, 18, D_IN], F32, tag="x_f32")
        xT = singles.tile([128, 2, TOK], BF16, tag="xT")
        qT = singles.tile([128, 2, TOK], BF16, tag="qT")
        kT = singles.tile([128, 2, TOK], BF16, tag="kT")
        # vT in slotted free layout: [dm-half part, half, frame, 32slot]
        vTs = singles.tile([128, 2, B2 * FR, 32], BF16, tag="vTs")
        # V in slot-token partitions, ones-augmented per head:
        # [128 slot-part, quad, half, h4, 33]
        vsl = singles.tile([128, NQ, 2, 4, DH + 1], BF16, tag="vsl")
        nc.vector.memset(vsl[:, :, :, :, DH:DH + 1], 1.0)
        out_sb = singles.tile([128, B2, QD, DM], F32, tag="out_sb")

        # ---- load x, cast to bf16 (overlapped with DMA), transpose ----
        x_view = x_d[:].rearrange("(t p) d -> p t d", p=128)
        for c in range(9):
            nc.sync.dma_start(
                x_f32[:, 2 * c:2 * (c + 1), :], x_view[:, 2 * c:2 * (c + 1), :]
            )
        x_bf = singles.tile([128, 18, D_IN], BF16, tag="x_bf")
        for t in range(18):
            eng = nc.vector if t % 2 else nc.scalar
            if eng is nc.scalar:
                nc.scalar.activation(
                    out=x_bf[:, t, :], in_=x_f32[:, t, :], func=AF.Copy
                )
            else:
                nc.vector.tensor_copy(x_bf[:, t, :], x_f32[:, t, :])
        # 4 bf16 transposes per PSUM bank, one batched copy per bank
        for t2 in range(9):
            tp = pstp.tile([128, 2, 2, 128], BF16, tag="xtp")
            for dt in range(2):
                for a in range(2):
                    nc.tensor.transpose(
                        tp[:, dt, a],
                        x_bf[:, 2 * t2 + dt, a * 128:(a + 1) * 128],
                        ident,
                    )
            nc.vector.tensor_copy(
                xT[:, :, 256 * t2:256 * (t2 + 1)].rearrange(
                    "p h (dt c) -> p dt h c", dt=2
                ),
                tp.rearrange("p dt h c -> p dt h c"),
            )

        # ---- projections (chunk-outer) with V transposes interleaved ----
        for c6 in range(NCH):
            c0 = c6 * CH
            for half in range(2):
                # q and k -> dense token layout, bias per-partition
                for di, (dst, wb, bn) in enumerate(
                    ((qT, wq_bf, "bq"), (kT, wk_bf, "bk"))
                ):
                    ps = psproj.tile([128, CH], F32, tag="proj")
                    for kk in range(2):
                        nc.tensor.matmul(
                            ps,
                            lhsT=wb[:, kk, half * 128:(half + 1) * 128],
                            rhs=xT[:, kk, c0:c0 + CH],
                            start=(kk == 0),
                            stop=(kk == 1),
                        )
                    if di == 0:
                        nc.scalar.activation(
                            out=dst[:, half, c0:c0 + CH],
                            in_=ps,
                            func=AF.Identity,
                            bias=b_sb[bn][:, half:half + 1],
                            scale=1.0,
                        )
                    else:
                        nc.vector.tensor_scalar_add(
                            dst[:, half, c0:c0 + CH],
                            ps,
                            b_sb[bn][:, half:half + 1],
                        )
                # v -> slotted vT, relu+bias fused
                ps = psproj.tile([128, CH], F32, tag="proj")
                for kk in range(2):
                    nc.tensor.matmul(
                        ps,
                        lhsT=wv_bf[:, kk, half * 128:(half + 1) * 128],
                        rhs=xT[:, kk, c0:c0 + CH],
                        start=(kk == 0),
                        stop=(kk == 1),
                    )
                f0 = c6 * 16
                nc.scalar.activation(
                    out=vTs[:, half, f0:f0 + 16, 0:J],
                    in_=ps.rearrange("p (f j) -> p f j", j=J),
                    func=AF.Relu,
                    bias=b_sb["bv"][:, half:half + 1],
                    scale=1.0,
                )
            # slot transposes for this chunk's 4 quads (2 per psum bank)
            for q2 in range(2):
                bq = 4 * c6 + 2 * q2
                vtp = pstp.tile([128, 2, 2, 128], BF16, tag="xtp")
                for dq in range(2):
                    for half in range(2):
                        nc.tensor.transpose(
                            vtp[:, dq, half],
                            vTs[:, half, 4 * (bq + dq):4 * (bq + dq) + 4, :],
                            ident,
                        )
                nc.vector.tensor_copy(
                    vsl[:, bq:bq + 2, :, :, 0:DH],
                    vtp.rearrange("p dq h (f d) -> p dq h f d", d=DH),
                )

        # ---- attention ----
        out_view = out_d[:].rearrange(
            "(b q s i) d -> b s i q d", b=B2, q=QD, s=4, i=J
        )
        # Software-pipelined: emit scores/exp for step i before AV/normalize
        # for step i-1 so the in-order PE queue never stalls on an exp.
        e_q = []
        for i in range(2 * H + 1):
            if i < 2 * H:
                b, h = divmod(i, H)
                half, h4 = h // 4, h % 4
                hr = 32 * h4
                sc = pssc.tile([128, QD, J], F32, tag="sc")
                if i < 2:
                    nc.vector.memset(sc, 0.0)  # init dead slot rows once/buffer
                for qd in range(QD):
                    for j in range(4):
                        t0 = b * N_TOK + qd * 96 + j * J
                        nc.tensor.matmul(
                            sc[32 * j:32 * j + J, qd, :],
                            lhsT=kT[hr:hr + 32, half, t0:t0 + J],
                            rhs=qT[hr:hr + 32, half, t0:t0 + J],
                            start=True,
                            stop=True,
                            tile_position=(hr, 32 * j),
                        )
                e = epool.tile([128, QD, J], BF16, tag="e")
                nc.scalar.activation(out=e, in_=sc, func=AF.Exp, scale=SCALE)
                e_q.append((b, h, e))
            if i >= 1:
                b, h, e = e_q[i - 1]
                half, h4 = h // 4, h % 4
                av = psav.tile([128, QD, DH + 1], F32, tag="av")
                if i <= 2:
                    nc.vector.memset(av, 0.0)
                for qd in range(QD):
                    for j in range(4):
                        nc.tensor.matmul(
                            av[32 * j:32 * j + J, qd, :],
                            lhsT=e[32 * j:32 * j + J, qd, :],
                            rhs=vsl[32 * j:32 * j + J, b * QD + qd, half, h4, :],
                            start=True,
                            stop=True,
                            tile_position=(32 * j, 32 * j),
                        )
                rt = rpool.tile([128, QD], F32, tag="rt")
                nc.vector.tensor_scalar_add(rt, av[:, :, DH], EPS)
                nc.vector.reciprocal(rt, rt)
                nc.vector.tensor_tensor(
                    out_sb[:, b, :, DH * h:DH * (h + 1)],
                    av[:, :, 0:DH],
                    rt[:, :, None].to_broadcast((128, QD, DH)),
                    ALU.mult,
                )
                if h == H - 1:
                    for s in range(4):
                        nc.sync.dma_start(
                            out_view[b, s], out_sb[32 * s:32 * s + J, b, :, :]
                        )

    nc.compile()
    return nc


def _get_nc():
    if "nc" not in _CACHE:
        _CACHE["nc"] = _build()
    return _CACHE["nc"]


def _run(inputs, **kw):
    nc = _get_nc()
    x = np.ascontiguousarray(inputs["x"], dtype=np.float32)
    shared = {
        k: np.ascontiguousarray(inputs[k], dtype=np.float32)
        for k in ("Wq", "Wk", "Wv", "bq", "bk", "bv")
    }
    in_maps = []
    for c in range(N_CORES):
        m = dict(shared)
        m["x"] = np.ascontiguousarray(x[c * B2:(c + 1) * B2].reshape(TOK, D_IN))
        in_maps.append(m)
    res = run_bass_kernel_spmd(nc, in_maps, core_ids=list(range(N_CORES)), **kw)
    out = np.concatenate(
        [r["out"].reshape(B2, N_TOK, DM) for r in res.results], axis=0
    )
    return out, res


def kernel(**inputs) -> np.ndarray:
    out, _ = _run(inputs)
    return out
